# revision 1
# baseline (speedup 1.0000x reference)
"""Causal self-attention Trainium2 kernel (B=2, T=2048, D=1024, H=16).

Sharding: 8 cores = 2 batch groups x 4 head groups; each core computes
batch b = c//4, heads 4*(c%4)..4*(c%4)+3 (256 QKV dims), and a partial
output projection y_cT = W_o[:, slice] @ attnout (contribution summed on
host across the 4 cores of each batch group).

All on-device compute in fp16 operands with fp32 PSUM accumulation.
Everything is kept "transposed" ([dim, seq]) so no on-device transposes
are needed:
  QT/KT = W @ xT               [256, 2048]
  V     = x @ WvT              [2048, 256]   (seq on partitions)
  ST[k,q] = sum_d K[k,d]Q[q,d] (k on partitions, q streaming)
  P = exp(ST/8); causal mask applied on diagonal 128x128 blocks
  avT[d,q] = sum_k [V|1][k,d] P[k,q]  -> row of ones gives softmax denom
  attnout[d,q] = avT * (1/denom) broadcast
  yT[e,q] = WoT.T @ attnout  (partial over this core's 256 dims)
"""

import numpy as np

import concourse.bass as bass
import concourse.mybir as mybir
from concourse.tile import TileContext
from concourse.vector_clock import ScopedClock
from concourse.bass_utils import run_bass_kernel_spmd

B, T, D = 2, 2048, 1024
H, DK = 16, 64
NCORES = 8
HPC = 4            # heads per core
QB = 512           # q block size
NQB = T // QB      # 4
NKC = T // 128     # 16 k-chunks
F16 = mybir.dt.float16
F32 = mybir.dt.float32
EXPF = mybir.ActivationFunctionType.Exp


class TC(TileContext):
    """This container's walrus only accepts one sync-wait per TPB_CTRL
    instruction; split the tile tail-drain waits into one nop each."""

    def _drain_and_barrier(self, tick_clock, wait_clock):
        carrier = self.nc.sync.nop(nofuse=True)
        wait_clock.add_sem_waits(
            carrier.ins, ScopedClock({None: tick_clock.global_clock})
        )
        si = carrier.ins.sync_info
        if si is not None and len(si.on_wait) > 1:
            waits = list(si.on_wait)
            carrier.ins.sync_info = mybir.SyncInfo(
                on_wait=[waits[0]], on_update=list(si.on_update)
            )
            for w in waits[1:]:
                nop = self.nc.sync.nop(nofuse=True)
                nop.ins.sync_info = mybir.SyncInfo(on_wait=[w], on_update=[])
        self.nc.sync.drain()
        self.nc.all_engine_barrier()
        assert self.sems is not None
        popped = self.nc._tile_sem_poison_stack.pop()
        assert popped is self._sem_poison
        self.nc.clear_and_free_semaphores(list(self.sems.allocated().values()))
        self.nc.all_engine_barrier()


def split_multi_waits(nc):
    """This walrus build accepts only one sync-wait per instruction; hoist
    extra waits onto single-wait NoOps inserted just before the instruction
    on the same engine."""
    for fn in nc.m.functions:
        for bb in fn.blocks:
            out = []
            for ins in bb.instructions:
                si = getattr(ins, "sync_info", None)
                is_isa = "ISA" in type(ins).__name__ or "PartitionBroadcast" in type(ins).__name__
                keep = 0 if is_isa else 1
                if si is not None and len(si.on_wait) > keep:
                    waits = list(si.on_wait)
                    keep_waits = waits[len(waits) - keep :] if keep else []
                    for i, w in enumerate(waits[: len(waits) - keep]):
                        out.append(
                            mybir.InstNoOp(
                                name=f"{ins.name}_w{i}",
                                engine=ins.engine,
                                sync_info=mybir.SyncInfo(on_wait=[w], on_update=[]),
                                bass_nofuse=True,
                            )
                        )
                    ins.sync_info = mybir.SyncInfo(
                        on_wait=keep_waits, on_update=list(si.on_update)
                    )
                out.append(ins)
            bb.instructions = out


def build_nc():
    nc = bass.Bass("TRN2", target_bir_lowering=False, debug=False)
    xT = nc.dram_tensor("xT", [D, T], F16, kind="ExternalInput")
    wqT = nc.dram_tensor("wqT", [D, 256], F16, kind="ExternalInput")
    wkT = nc.dram_tensor("wkT", [D, 256], F16, kind="ExternalInput")
    wvT = nc.dram_tensor("wvT", [D, 256], F16, kind="ExternalInput")
    woT = nc.dram_tensor("woT", [256, D], F16, kind="ExternalInput")
    tmask = nc.dram_tensor("tmask", [128, 128], F16, kind="ExternalInput")
    sel65 = nc.dram_tensor("sel65", [65, 128], F16, kind="ExternalInput")
    yT = nc.dram_tensor("yT", [D, T], F16, kind="ExternalOutput")

    with TC(nc) as tc:
        with (
            tc.tile_pool(name="const", bufs=1) as cpool,
            tc.tile_pool(name="work", bufs=2) as wpool,
            tc.tile_pool(name="psA", bufs=2, space="PSUM") as psA,
            tc.tile_pool(name="psS", bufs=2, space="PSUM") as psS,
            tc.tile_pool(name="psV", bufs=1, space="PSUM") as psV,
        ):
            # ---- load inputs ----
            xt = []
            for kc in range(8):
                t = cpool.tile([128, T], F16, tag=f"xt{kc}")
                nc.sync.dma_start(t[:], xT[128 * kc : 128 * (kc + 1), :])
                xt.append(t)
            wq, wk, wv = [], [], []
            for name, dram, lst in (("wq", wqT, wq), ("wk", wkT, wk), ("wv", wvT, wv)):
                for kc in range(8):
                    t = cpool.tile([128, 256], F16, tag=f"{name}{kc}")
                    nc.sync.dma_start(t[:], dram[128 * kc : 128 * (kc + 1), :])
                    lst.append(t)
            wo = []
            for p in range(2):
                t = cpool.tile([128, D], F16, tag=f"wo{p}")
                nc.sync.dma_start(t[:], woT[128 * p : 128 * (p + 1), :])
                wo.append(t)
            mask = cpool.tile([128, 128], F16, tag="mask")
            nc.sync.dma_start(mask[:], tmask[:, :])
            sel = cpool.tile([65, 128], F16, tag="sel")
            nc.sync.dma_start(sel[:], sel65[:, :])

            qt = [cpool.tile([128, T], F16, tag=f"qt{p}", name=f"qt{p}") for p in range(2)]
            kt = [cpool.tile([128, T], F16, tag=f"kt{p}", name=f"kt{p}") for p in range(2)]
            ao = [cpool.tile([128, T], F16, tag=f"ao{p}", name=f"ao{p}") for p in range(2)]

            # ---- Q, K projections: out[p][:, jq] = W.T @ xT ----
            # pair 0 first so attention(p0) can start while pair-1
            # projections and V are still running
            def qk_proj(p, wt, out_t, jq):
                ps = psA.tile([128, QB], F32, tag="psA", name=f"psqk{p}{jq}")
                for kc in range(8):
                    nc.tensor.matmul(
                        ps[:],
                        wt[kc][:, 128 * p : 128 * (p + 1)],
                        xt[kc][:, QB * jq : QB * (jq + 1)],
                        start=(kc == 0),
                        stop=(kc == 7),
                    )
                nc.vector.tensor_copy(out_t[p][:, QB * jq : QB * (jq + 1)], ps[:])
            vp = [
                [cpool.tile([128, 193], F16, tag=f"vp{tt}_{p}", name=f"vp{tt}_{p}") for p in range(2)]
                for tt in range(NKC)
            ]

            def v_proj(tt):
                ps = psA.tile([128, QB], F32, tag="psA", name=f"psv{tt}")
                for kc in range(8):
                    nc.tensor.matmul(
                        ps[:, 0:256],
                        xt[kc][:, 128 * tt : 128 * (tt + 1)],
                        wv[kc][:],
                        start=(kc == 0),
                        stop=(kc == 7),
                    )
                for p in range(2):
                    v = vp[tt][p]
                    nc.vector.memset(v[:, 64:66], 1.0)
                    nc.vector.memset(v[:, 66:129], 0.0)
                    nc.vector.tensor_copy(v[:, 0:64], ps[:, 128 * p : 128 * p + 64])
                    nc.vector.tensor_copy(v[:, 129:193], ps[:, 128 * p + 64 : 128 * p + 128])

            # ---- pipeline: QK(jq)/V blocks interleave with attention ----
            def attention(j, units=()):
                units = list(units)
                nch = 4 * j + 4
                for p in range(2):
                    P = wpool.tile([128, 1024 * NKC], F16, tag="P", bufs=2)
                    av0 = psV.tile([65, QB], F32, tag="av0")
                    av1 = psV.tile([128, QB], F32, tag="av1")
                    for kc in range(nch):
                        off = max(0, 128 * (kc - 4 * j))
                        ps = psS.tile([128, 1024], F32, tag="psS")
                        for h in range(2):
                            nc.tensor.matmul(
                                ps[:, 512 * h + off : 512 * (h + 1)],
                                kt[p][64 * h : 64 * (h + 1), 128 * kc : 128 * (kc + 1)],
                                qt[p][64 * h : 64 * (h + 1), QB * j + off : QB * (j + 1)],
                                start=True,
                                stop=True,
                            )
                        nc.scalar.activation(
                            P[:, 1024 * kc + off : 1024 * (kc + 1)],
                            ps[:, off:1024],
                            EXPF,
                            scale=0.125,
                        )
                        if kc >= 4 * j:  # diagonal band: mask 128x128 blocks
                            for h in range(2):
                                sl = slice(
                                    1024 * kc + 512 * h + off,
                                    1024 * kc + 512 * h + off + 128,
                                )
                                nc.vector.tensor_mul(P[:, sl], P[:, sl], mask[:])
                        nc.tensor.matmul(
                            av0[:, off:QB],
                            vp[kc][p][:, 0:65],
                            P[:, 1024 * kc + off : 1024 * kc + 512],
                            start=(kc == 0),
                            stop=(kc == nch - 1),
                        )
                        nc.tensor.matmul(
                            av1[:, off:QB],
                            vp[kc][p][:, 65:193],
                            P[:, 1024 * kc + 512 + off : 1024 * (kc + 1)],
                            start=(kc == 0),
                            stop=(kc == nch - 1),
                        )
                        step = p * nch + kc + 1
                        target = len(units) * step // (2 * nch)
                        while getattr(attention, "_emitted", 0) < target:
                            units[getattr(attention, "_emitted", 0)]()
                            attention._emitted = getattr(attention, "_emitted", 0) + 1
                    # softmax denominators: av0 row 64 = sum(P_h0),
                    # av1 row 0 = sum(P_h1); av1 values live on rows 64..127
                    rcb = wpool.tile([65, QB], F16, tag="rcb", bufs=2)
                    nc.vector.memset(rcb[0:64, :], 0.0)
                    with nc.allow_low_precision("softmax denom recip in fp16"):
                        nc.vector.reciprocal(rcb[64:65, :], av0[64:65, :])
                        nc.vector.reciprocal(rcb[0:1, :], av1[0:1, :])
                    bc = psA.tile([128, QB], F32, tag="psA")
                    nc.tensor.matmul(bc[:], sel[:], rcb[:], start=True, stop=True)
                    bcs = wpool.tile([128, QB], F32, tag="bcs", bufs=2)
                    nc.vector.tensor_copy(bcs[:], bc[:])
                    nc.vector.tensor_mul(
                        ao[p][0:64, QB * j : QB * (j + 1)], av0[0:64, :], bcs[0:64, :]
                    )
                    nc.vector.tensor_mul(
                        ao[p][64:128, QB * j : QB * (j + 1)],
                        av1[64:128, :],
                        bcs[64:128, :],
                    )
                for u in units[getattr(attention, "_emitted", 0):]:
                    u()
                attention._emitted = 0
                # out-projection for this q block
                for et in range(8):
                    ps = psA.tile([128, QB], F32, tag="psA")
                    for p in range(2):
                        nc.tensor.matmul(
                            ps[:],
                            wo[p][:, 128 * et : 128 * (et + 1)],
                            ao[p][:, QB * j : QB * (j + 1)],
                            start=(p == 0),
                            stop=(p == 1),
                        )
                    ysb = wpool.tile([128, QB], F16, tag="ysb", bufs=3)
                    nc.vector.tensor_copy(ysb[:], ps[:])
                    nc.sync.dma_start(
                        yT[128 * et : 128 * (et + 1), QB * j : QB * (j + 1)], ysb[:]
                    )

            # interleaved schedule: projections for block jq+1 and V rows for
            # block j+1 are generated right after attention(j) so the
            # scheduler overlaps PE-dense projection work with the ACT-bound
            # attention phase.
            for pp, wt, ot in ((0, wq, qt), (1, wq, qt), (0, wk, kt), (1, wk, kt)):
                qk_proj(pp, wt, ot, 0)
            for tt in range(4):
                v_proj(tt)
            for j in range(NQB):
                units = []
                if j + 1 < NQB:
                    jq = j + 1
                    for pp, wt, ot in ((0, wq, qt), (1, wq, qt), (0, wk, kt), (1, wk, kt)):
                        units.append(lambda pp=pp, wt=wt, ot=ot, jq=jq: qk_proj(pp, wt, ot, jq))
                    for tt in range(4 * jq, 4 * jq + 4):
                        units.append(lambda tt=tt: v_proj(tt))
                attention(j, units)
    split_multi_waits(nc)
    return nc


_NC = None


def _get_nc():
    global _NC
    if _NC is None:
        _NC = build_nc()
    return _NC


def kernel(x, W_q, W_k, W_v, W_o):
    x = np.asarray(x, dtype=np.float32)
    W_q = np.asarray(W_q, dtype=np.float32)
    W_k = np.asarray(W_k, dtype=np.float32)
    W_v = np.asarray(W_v, dtype=np.float32)
    W_o = np.asarray(W_o, dtype=np.float32)

    tmask = np.triu(np.ones((128, 128), dtype=np.float16))
    sel65 = np.zeros((65, 128), dtype=np.float16)
    sel65[64, 0:64] = 1.0
    sel65[0, 64:128] = 1.0
    xTb = [np.ascontiguousarray(x[b].T).astype(np.float16) for b in range(B)]
    in_maps = []
    for c in range(NCORES):
        b, g = c // 4, c % 4
        hs = 256 * g
        in_maps.append(
            {
                "xT": xTb[b],
                "wqT": np.ascontiguousarray(W_q[hs : hs + 256, :].T).astype(np.float16),
                "wkT": np.ascontiguousarray(W_k[hs : hs + 256, :].T).astype(np.float16),
                "wvT": np.ascontiguousarray(W_v[hs : hs + 256, :].T).astype(np.float16),
                "woT": np.ascontiguousarray(W_o[:, hs : hs + 256].T).astype(np.float16),
                "tmask": tmask,
                "sel65": sel65,
            }
        )
    res = run_bass_kernel_spmd(_get_nc(), in_maps, core_ids=list(range(NCORES)))
    out = np.empty((B, T, D), dtype=np.float32)
    for b in range(B):
        acc = res.results[4 * b]["yT"].astype(np.float32)
        for g in range(1, 4):
            acc = acc + res.results[4 * b + g]["yT"]
        out[b] = acc.T
    return out



# revision 30
# speedup vs baseline: 1.2802x; 1.2802x over previous
"""Causal self-attention Trainium2 kernel (B=2, T=2048, D=1024, H=16).

Sharding: 8 cores = 2 batch groups x 4 head groups; each core computes
batch b = c//4, heads 4*(c%4)..4*(c%4)+3 (256 QKV dims), and a partial
output projection y_cT = W_o[:, slice] @ attnout (contribution summed on
host across the 4 cores of each batch group).

QKV projections run in fp8e4 DoubleRow (0.5 cyc/row, 256-contraction)
with a hi/lo residual split of both x and W (3 accumulated terms:
hi*hi + lo*hi + hi*lo), computed on host with power-of-2 scales
(x*2^4, W*2^9) so values sit mid-range in e4m3.  The resulting 2^13
scale on Q/K folds into the softmax exp scale (2^-26/8); the 2^13 on V
propagates through AV and the out-projection and is divided off on the
host.  Attention (QK^T, exp, AV) and the out-projection stay fp16.

Layout is fully "transposed" ([dim, seq]) so no on-device transposes:
  QT/KT = W @ xT               [256, 2048]  (x2^13)
  V     = x @ WvT              [2048, 256]  (x2^13, seq on partitions)
  ST[k,q] = sum_d K[k,d]Q[q,d] (k on partitions, q streaming)
  P = exp(ST * 2^-26/8); causal mask (Pool engine) on diagonal blocks
  avT[d,q] = sum_k [V|1][k,d] P[k,q]  -> unscaled ones row = denom
  attnout[d,q] = avT * (1/denom) broadcast
  yT[e,q] = WoT.T @ attnout  (partial over this core's 256 dims)
"""

import numpy as np

import concourse.bass as bass
import concourse.mybir as mybir
from concourse.tile import TileContext
from concourse.vector_clock import ScopedClock
from concourse.bass_utils import run_bass_kernel_spmd

B, T, D = 2, 2048, 1024
H, DK = 16, 64
NCORES = 8
HPC = 4            # heads per core
QB = 512           # q block size
NQB = T // QB      # 4
NKC = T // 128     # 16 k-chunks
F16 = mybir.dt.float16
F32 = mybir.dt.float32
F8 = mybir.dt.float8e4
NP8 = mybir.dt.np(F8)
DR = mybir.MatmulPerfMode.DoubleRow
EXPF = mybir.ActivationFunctionType.Exp
SX = 16.0          # x pre-scale (2^4)
SW = 512.0         # W_{q,k,v} pre-scale (2^9)
YS = SX * SW       # 2^13: scale carried by Q', K', V', ao', y'
ESCALE = 0.125 / (YS * YS)   # exp scale: undo Q'K' 2^26 and /sqrt(dk)
QK8 = True         # QK^T in fp8 DoubleRow: K hi/lo-corrected, Q pure fp8
SQK = 16.0         # Q/K fp8 store scale (2^4)
ESCALE_QK = 0.125 / (SQK * SQK)


class TC(TileContext):
    """This container's walrus only accepts one sync-wait per TPB_CTRL
    instruction; split the tile tail-drain waits into one nop each."""

    def _drain_and_barrier(self, tick_clock, wait_clock):
        carrier = self.nc.sync.nop(nofuse=True)
        wait_clock.add_sem_waits(
            carrier.ins, ScopedClock({None: tick_clock.global_clock})
        )
        si = carrier.ins.sync_info
        if si is not None and len(si.on_wait) > 1:
            waits = list(si.on_wait)
            carrier.ins.sync_info = mybir.SyncInfo(
                on_wait=[waits[0]], on_update=list(si.on_update)
            )
            for w in waits[1:]:
                nop = self.nc.sync.nop(nofuse=True)
                nop.ins.sync_info = mybir.SyncInfo(on_wait=[w], on_update=[])
        self.nc.sync.drain()
        self.nc.all_engine_barrier()
        assert self.sems is not None
        popped = self.nc._tile_sem_poison_stack.pop()
        assert popped is self._sem_poison
        self.nc.clear_and_free_semaphores(list(self.sems.allocated().values()))
        self.nc.all_engine_barrier()


def split_multi_waits(nc):
    """This walrus build accepts only one sync-wait per instruction; hoist
    extra waits onto single-wait NoOps inserted just before the instruction
    on the same engine."""
    for fn in nc.m.functions:
        for bb in fn.blocks:
            out = []
            for ins in bb.instructions:
                si = getattr(ins, "sync_info", None)
                is_isa = "ISA" in type(ins).__name__ or "PartitionBroadcast" in type(ins).__name__
                keep = 0 if is_isa else 1
                if si is not None and len(si.on_wait) > keep:
                    waits = list(si.on_wait)
                    keep_waits = waits[len(waits) - keep :] if keep else []
                    for i, w in enumerate(waits[: len(waits) - keep]):
                        out.append(
                            mybir.InstNoOp(
                                name=f"{ins.name}_w{i}",
                                engine=ins.engine,
                                sync_info=mybir.SyncInfo(on_wait=[w], on_update=[]),
                                bass_nofuse=True,
                            )
                        )
                    ins.sync_info = mybir.SyncInfo(
                        on_wait=keep_waits, on_update=list(si.on_update)
                    )
                out.append(ins)
            bb.instructions = out


def build_nc():
    nc = bass.Bass("TRN2", target_bir_lowering=False, debug=False)
    # x packed [jq][k][g][i][c]: d = 256g + 128i + k, t = 512jq + c
    xh = nc.dram_tensor("xh", [NQB, 128, 4, 2, QB], F8, kind="ExternalInput")
    xl = nc.dram_tensor("xl", [NQB, 128, 4, 2, QB], F8, kind="ExternalInput")
    # W packed [k][g][i][m]
    wts = {}
    for w in ("wqh", "wql", "wkh", "wkl", "wvh", "wvl"):
        wts[w] = nc.dram_tensor(w, [128, 4, 2, 256], F8, kind="ExternalInput")
    woT = nc.dram_tensor("woT", [256, D], F16, kind="ExternalInput")
    tmask = nc.dram_tensor("tmask", [128, 128], F16, kind="ExternalInput")
    sel65 = nc.dram_tensor("sel65", [65, 128], F16, kind="ExternalInput")
    yT = nc.dram_tensor("yT", [D, T], F16, kind="ExternalOutput")

    with TC(nc) as tc:
        with (
            tc.tile_pool(name="const", bufs=1) as cpool,
            tc.tile_pool(name="work", bufs=2) as wpool,
            tc.tile_pool(name="psA", bufs=2, space="PSUM") as psA,
            tc.tile_pool(name="psS", bufs=2, space="PSUM") as psS,
            tc.tile_pool(name="psV", bufs=1, space="PSUM") as psV,
        ):
            # ---- tiles ----
            xht = [cpool.tile([128, 4, 2, QB], F8, tag=f"xht{j}", name=f"xht{j}") for j in range(NQB)]
            xlt = [cpool.tile([128, 4, 2, QB], F8, tag=f"xlt{j}", name=f"xlt{j}") for j in range(NQB)]
            wt = {
            w: cpool.tile([128, 4, 2, 256], F8, tag=w, name=w)
                for w in ("wqh", "wql", "wkh", "wkl", "wvh", "wvl")
            }
            wo = [cpool.tile([128, D], F16, tag=f"wo{p}", name=f"wo{p}") for p in range(2)]
            mask = cpool.tile([128, 128], F16, tag="mask")
            sel = cpool.tile([65, 128], F16, tag="sel")

            # ---- input DMAs, priority order: attention(0) prereqs first ----
            nc.sync.dma_start(wt["wqh"][:], wts["wqh"][:, :, :, :])
            nc.sync.dma_start(xht[0][:], xh[0, :, :, :, :])
            nc.sync.dma_start(wt["wql"][:], wts["wql"][:, :, :, :])
            nc.sync.dma_start(xlt[0][:], xl[0, :, :, :, :])
            nc.sync.dma_start(wt["wkh"][:], wts["wkh"][:, :, :, :])
            nc.sync.dma_start(wt["wkl"][:], wts["wkl"][:, :, :, :])
            nc.sync.dma_start(wt["wvh"][:], wts["wvh"][:, :, :, :])
            nc.sync.dma_start(wt["wvl"][:], wts["wvl"][:, :, :, :])
            nc.sync.dma_start(mask[:], tmask[:, :])
            nc.sync.dma_start(sel[:], sel65[:, :])
            nc.sync.dma_start(xht[1][:], xh[1, :, :, :, :])
            nc.sync.dma_start(xlt[1][:], xl[1, :, :, :, :])
            for p in range(2):
                nc.sync.dma_start(wo[p][:], woT[128 * p : 128 * (p + 1), :])
            for j in (2, 3):
                nc.sync.dma_start(xht[j][:], xh[j, :, :, :, :])
                nc.sync.dma_start(xlt[j][:], xl[j, :, :, :, :])

            if QK8:
                # dim1: Q duplicated (pure fp8); K split (hi, lo residual)
                qt = [cpool.tile([128, 2, T], F8, tag=f"qt{p}", name=f"qt{p}") for p in range(2)]
                kt = [cpool.tile([128, 2, T], F8, tag=f"kt{p}", name=f"kt{p}") for p in range(2)]
            else:
                qt = [cpool.tile([128, T], F16, tag=f"qt{p}", name=f"qt{p}") for p in range(2)]
                kt = [cpool.tile([128, T], F16, tag=f"kt{p}", name=f"kt{p}") for p in range(2)]
            ao = [cpool.tile([128, T], F16, tag=f"ao{p}", name=f"ao{p}") for p in range(2)]

            # vp[tt][p]: [V_h0 (0:64) | ones (64) | zeros (65:128) | V_h1 (128:192) | pad]
            # av0 lhsT = vp[:, 0:65]  -> rows 0:64 V_h0, row 64 = denom_h0
            # av1 lhsT = vp[:, 64:192]-> row 0 = denom_h1, rows 64:128 V_h1
            vp = [
                [cpool.tile([128, 256], F16, tag=f"vp{tt}_{p}", name=f"vp{tt}_{p}") for p in range(2)]
                for tt in range(NKC)
            ]
            for tt in range(NKC):
                for p in range(2):
                    nc.gpsimd.memset(vp[tt][p][:, 64:65], 1.0)
                    nc.gpsimd.memset(vp[tt][p][:, 65:128], 0.0)

            # ---- fp8 DoubleRow projections ----
            def qk_proj(p, hi, lo, out_t, jq):
                is_k = hi == "wkh"
                ps = psA.tile([128, QB], F32, tag="psA", name=f"psqk{p}{jq}")
                terms = (
                    [(wt[hi], xht[jq], g) for g in range(4)]
                    + [(wt[lo], xht[jq], g) for g in range(4)]
                    + [(wt[hi], xlt[jq], g) for g in range(4)]
                )
                for i, (w, x, g) in enumerate(terms):
                    nc.tensor.matmul(
                        ps[:],
                        w[:, g, :, 128 * p : 128 * (p + 1)],
                        x[:, g, :, :],
                        start=(i == 0),
                        stop=(i == len(terms) - 1),
                        perf_mode=DR,
                    )
                if not QK8:
                    nc.vector.tensor_copy(out_t[p][:, QB * jq : QB * (jq + 1)], ps[:])
                    return
                cols = slice(QB * jq, QB * (jq + 1))
                if is_k:
                    ktf = wpool.tile([128, QB], F16, tag="ktf", bufs=2, name=f"ktf{p}{jq}")
                    nc.vector.tensor_scalar_mul(ktf[:], ps[:], 1.0 / 512.0)
                    nc.vector.tensor_copy(out_t[p][:, 0, cols], ktf[:])
                    with nc.allow_low_precision("fp8 residual split of K"):
                        nc.vector.tensor_sub(
                            out_t[p][:, 1, cols], ktf[:], out_t[p][:, 0, cols]
                        )
                else:
                    with nc.allow_low_precision("Q stored pure fp8"):
                        nc.vector.tensor_scalar_mul(
                            out_t[p][:, 0, cols], ps[:], 1.0 / 512.0
                        )
                        nc.vector.tensor_copy(out_t[p][:, 1, cols], out_t[p][:, 0, cols])

            def v_proj(tt):
                jq, tloc = tt // 4, tt % 4
                ps = psA.tile([128, QB], F32, tag="psA", name=f"psv{tt}")
                terms = (
                    [(xht[jq], wt["wvh"], g) for g in range(4)]
                    + [(xlt[jq], wt["wvh"], g) for g in range(4)]
                    + [(xht[jq], wt["wvl"], g) for g in range(4)]
                )
                for i, (x, w, g) in enumerate(terms):
                    nc.tensor.matmul(
                        ps[:, 0:256],
                        x[:, g, :, 128 * tloc : 128 * (tloc + 1)],
                        w[:, g, :, :],
                        start=(i == 0),
                        stop=(i == len(terms) - 1),
                        perf_mode=DR,
                    )
                for p in range(2):
                    v = vp[tt][p]
                    src = ps[:, 128 * p : 128 * (p + 1)]
                    dst = v[:, 0:256].rearrange("p (a b) -> p a b", b=128)[:, :, 0:64]
                    nc.vector.tensor_copy(
                        dst, src.rearrange("p (a b) -> p a b", b=64)
                    )

            # out-projection for one q block, emitted as 8 per-et units.
            # act_copy spreads the PSUM->SBUF copy onto the (by then idle)
            # Activation engine so the final block's copies don't serialize
            # on DVE in the kernel tail.
            def out_proj_unit(j, et, act_copy=False, borrow_av=False):
                if borrow_av:
                    # after the last norm, psV's av1 bank is free: use it as a
                    # third rotation slot so the tail's copies pipeline deeper
                    ps = psV.tile([128, QB], F32, tag="av1", name=f"psb{j}{et}")
                else:
                    ps = psA.tile([128, QB], F32, tag="psA")
                for p in range(2):
                    nc.tensor.matmul(
                        ps[:],
                        wo[p][:, 128 * et : 128 * (et + 1)],
                        ao[p][:, QB * j : QB * (j + 1)],
                        start=(p == 0),
                        stop=(p == 1),
                    )
                ysb = wpool.tile([128, QB], F16, tag="ysb", bufs=3)
                if act_copy:
                    nc.scalar.activation(
                        ysb[:], ps[:], mybir.ActivationFunctionType.Copy, scale=1.0
                    )
                else:
                    nc.vector.tensor_copy(ysb[:], ps[:])
                nc.sync.dma_start(
                    yT[128 * et : 128 * (et + 1), QB * j : QB * (j + 1)], ysb[:]
                )

            # ---- pipeline: fillers interleave with ACT-paced attention ----
            def attention(j, units=()):
                units = list(units)
                nch = 4 * j + 4

                def av_mm(p, av0, av1, kc):
                    off = max(0, 128 * (kc - 4 * j))
                    P = pcur[p]
                    nc.tensor.matmul(
                        av0[:, off:QB],
                        vp[kc][p][:, 0:65],
                        P[:, 1024 * kc + off : 1024 * kc + 512],
                        start=(kc == 0),
                        stop=(kc == nch - 1),
                    )
                    nc.tensor.matmul(
                        av1[:, off:QB],
                        vp[kc][p][:, 64:192],
                        P[:, 1024 * kc + 512 + off : 1024 * (kc + 1)],
                        start=(kc == 0),
                        stop=(kc == nch - 1),
                    )

                steps_total = 2 * (nch + 2)

                def pace(step):
                    target = len(units) * step // steps_total
                    while getattr(attention, "_emitted", 0) < target:
                        units[getattr(attention, "_emitted", 0)]()
                        attention._emitted = getattr(attention, "_emitted", 0) + 1

                def norm(p, av0, av1):
                    # softmax denominators: av0 row 64 = sum(P_h0),
                    # av1 row 0 = sum(P_h1); av1 values live on rows 64..127
                    rcb = wpool.tile([65, QB], F16, tag="rcb", bufs=2, name=f"rcb{j}{p}")
                    if j == 0:
                        nc.vector.memset(rcb[0:64, :], 0.0)
                    with nc.allow_low_precision("softmax denom recip in fp16"):
                        nc.vector.reciprocal(rcb[64:65, :], av0[64:65, :])
                        nc.vector.reciprocal(rcb[0:1, :], av1[0:1, :])
                    bc = psA.tile([128, QB], F32, tag="psA", name=f"bc{j}{p}")
                    nc.tensor.matmul(bc[:], sel[:], rcb[:], start=True, stop=True)
                    bcs = wpool.tile([128, QB], F32, tag="bcs", bufs=2, name=f"bcs{j}{p}")
                    nc.vector.tensor_copy(bcs[:], bc[:])
                    nc.vector.tensor_mul(
                        ao[p][0:64, QB * j : QB * (j + 1)], av0[0:64, :], bcs[0:64, :]
                    )
                    nc.vector.tensor_mul(
                        ao[p][64:128, QB * j : QB * (j + 1)],
                        av1[64:128, :],
                        bcs[64:128, :],
                    )

                pcur = {}
                for p in range(2):
                    pcur[p] = wpool.tile([128, 1024 * NKC], F16, tag="P", bufs=2, name=f"P{j}{p}")
                    av0 = psV.tile([65, QB], F32, tag="av0", name=f"av0{j}{p}")
                    av1 = psV.tile([128, QB], F32, tag="av1", name=f"av1{j}{p}")
                    P = pcur[p]
                    for kc in range(nch):
                        off = max(0, 128 * (kc - 4 * j))
                        ps = psS.tile([128, 1024], F32, tag="psS", name=f"ps{j}{p}{kc}")
                        for h in range(2):
                            if QK8:
                                nc.tensor.matmul(
                                    ps[:, 512 * h + off : 512 * (h + 1)],
                                    kt[p][64 * h : 64 * (h + 1), :, 128 * kc : 128 * (kc + 1)],
                                    qt[p][64 * h : 64 * (h + 1), :, QB * j + off : QB * (j + 1)],
                                    start=True,
                                    stop=True,
                                    perf_mode=DR,
                                )
                            else:
                                nc.tensor.matmul(
                                    ps[:, 512 * h + off : 512 * (h + 1)],
                                    kt[p][64 * h : 64 * (h + 1), 128 * kc : 128 * (kc + 1)],
                                    qt[p][64 * h : 64 * (h + 1), QB * j + off : QB * (j + 1)],
                                    start=True,
                                    stop=True,
                                )
                        nc.scalar.activation(
                            P[:, 1024 * kc + off : 1024 * (kc + 1)],
                            ps[:, off:1024],
                            EXPF,
                            scale=(ESCALE_QK if QK8 else ESCALE),
                        )
                        if kc >= 4 * j:  # diagonal band: mask 128x128 blocks
                            for h in range(2):
                                sl = slice(
                                    1024 * kc + 512 * h + off,
                                    1024 * kc + 512 * h + off + 128,
                                )
                                nc.gpsimd.tensor_mul(P[:, sl], P[:, sl], mask[:])
                        # the previous pair's norm chain is deferred to here:
                        # its recips run while this pair's first QK/exps fill PE
                        if kc == 1 and attention._pending_norm is not None:
                            attention._pending_norm()
                            attention._pending_norm = None
                        # AV trails QK/exp by one chunk so the in-order PE
                        # queue never waits on a just-issued exp
                        if kc > 0:
                            av_mm(p, av0, av1, kc - 1)
                        pace(p * (nch + 2) + kc + 1)
                    # boundary stalls: fill before the trailing AV (waits the
                    # final exp) and before the norm (waits the DVE recips)
                    pace(p * (nch + 2) + nch + 1)
                    av_mm(p, av0, av1, nch - 1)
                    pace(p * (nch + 2) + nch + 2)
                    attention._pending_norm = lambda p=p, a0=av0, a1=av1: norm(p, a0, a1)
                for u in units[getattr(attention, "_emitted", 0):]:
                    u()
                attention._emitted = 0

            # prolog: projections for q block 0 / k chunks 0..3
            for pp, hi, lo, ot in (
                (0, "wqh", "wql", qt),
                (0, "wkh", "wkl", kt),
                (1, "wqh", "wql", qt),
                (1, "wkh", "wkl", kt),
            ):
                qk_proj(pp, hi, lo, ot, 0)
            for tt in range(4):
                v_proj(tt)
            # filler assignment keeps late (filler-poor) attention phases fed:
            # att(j) gets projections for block j+1; out-projs lag two blocks
            # (att(2): outproj(0); att(3): outproj(1)+outproj(2)).
            # phase order and filler assignment: j=3 (the heaviest, most
            # exp-bound block) runs third so it has projections AND early
            # out-projections as PE filler; the light j=2 closes.
            def qk_units(jq):
                us = []
                for pp, hi, lo, ot in (
                    (0, "wqh", "wql", qt),
                    (0, "wkh", "wkl", kt),
                    (1, "wqh", "wql", qt),
                    (1, "wkh", "wkl", kt),
                ):
                    us.append(
                        lambda pp=pp, hi=hi, lo=lo, ot=ot, jq=jq: qk_proj(pp, hi, lo, ot, jq)
                    )
                return us

            def v_units(jq):
                return [
                    (lambda tt=tt: v_proj(tt)) for tt in range(4 * jq, 4 * jq + 4)
                ]

            def op_units(jj, ets, act=False):
                return [
                    (lambda jj=jj, et=et, act=act: out_proj_unit(jj, et, act_copy=act))
                    for et in ets
                ]

            # (j, eager units emitted before the kc loop, paced units)
            plan = [
                (0, [], qk_units(1) + v_units(1)),
                (1, [], qk_units(2) + v_units(2)),
                (2, [], qk_units(3) + v_units(3)),
                (3, [], op_units(0, range(8)) + op_units(1, range(8)) + op_units(2, range(8))),
            ]
            attention._pending_norm = None
            for j, eager, paced in plan:
                for u in eager:
                    u()
                attention(j, paced)
            if attention._pending_norm is not None:
                attention._pending_norm()
                attention._pending_norm = None
            for et in range(8):
                out_proj_unit(NQB - 1, et, borrow_av=(et % 3 == 2))

    split_multi_waits(nc)
    return nc


_NC = None


def _get_nc():
    global _NC
    if _NC is None:
        _NC = build_nc()
    return _NC


def _split8(a):
    hi = a.astype(NP8)
    lo = (a - hi.astype(np.float32)).astype(NP8)
    return hi, lo


def kernel(x, W_q, W_k, W_v, W_o):
    x = np.asarray(x, dtype=np.float32)
    W_q = np.asarray(W_q, dtype=np.float32)
    W_k = np.asarray(W_k, dtype=np.float32)
    W_v = np.asarray(W_v, dtype=np.float32)
    W_o = np.asarray(W_o, dtype=np.float32)

    tmask = np.triu(np.ones((128, 128), dtype=np.float16))
    sel65 = np.zeros((65, 128), dtype=np.float16)
    sel65[64, 0:64] = 1.0
    sel65[0, 64:128] = 1.0

    # x packed [jq][k][g][i][c]: d = 256g + 128i + k, t = 512jq + c
    xpk = []
    for b in range(B):
        xT = np.ascontiguousarray(x[b].T) * SX          # [D, T]
        arr = xT.reshape(4, 2, 128, NQB, QB)            # [g, i, k, jq, c]
        arr = np.ascontiguousarray(arr.transpose(3, 2, 0, 1, 4))  # [jq,k,g,i,c]
        xpk.append(_split8(arr))

    def packw(Wslice):  # [256 out, 1024 in] -> hi/lo [k][g][i][m]
        wT = np.ascontiguousarray(Wslice.T) * SW        # [1024, 256]
        arr = wT.reshape(4, 2, 128, 256).transpose(2, 0, 1, 3)
        return _split8(np.ascontiguousarray(arr))

    in_maps = []
    for c in range(NCORES):
        b, g = c // 4, c % 4
        hs = 256 * g
        wqh, wql = packw(W_q[hs : hs + 256, :])
        wkh, wkl = packw(W_k[hs : hs + 256, :])
        wvh, wvl = packw(W_v[hs : hs + 256, :])
        in_maps.append(
            {
                "xh": xpk[b][0],
                "xl": xpk[b][1],
                "wqh": wqh,
                "wql": wql,
                "wkh": wkh,
                "wkl": wkl,
                "wvh": wvh,
                "wvl": wvl,
                "woT": np.ascontiguousarray(W_o[:, hs : hs + 256].T).astype(np.float16),
                "tmask": tmask,
                "sel65": sel65,
            }
        )
    res = run_bass_kernel_spmd(_get_nc(), in_maps, core_ids=list(range(NCORES)))
    out = np.empty((B, T, D), dtype=np.float32)
    for b in range(B):
        acc = res.results[4 * b]["yT"].astype(np.float32)
        for g in range(1, 4):
            acc = acc + res.results[4 * b + g]["yT"]
        out[b] = acc.T / YS
    return out
